# revision 33
# baseline (speedup 1.0000x reference)
import sys

sys.path.insert(0, "/opt/trn_rl_repo")
import numpy as np

# nn_BisineNetwork: out[n,c] = sum_k a[c,k] * sin(x@w1[c,k]+b1[c,k]) * sin(x@w2[c,k]+b2[c,k])
# Shapes (hardcoded): x (16384, 256) f32, params (1000, 2060) f32 -> out (16384, 1000) f32.
#
# Sharding: data-parallel over batch N across 8 cores (N_shard = 2048); params
# replicated. Per-core layout is [ck, n] (c,k merged -> 4000, padded to 4096).
# W is pre-scaled by 1/2pi on host so u arrives in "turns":
#   u1 = W1blk.T @ Xshard          (PE fp16, contraction d=256 in 2 chunks, psum f32)
#   m1 = wrap(u1 + b1') in [-.5,.5] (custom DVE op: magic-number round, 1 pass)
#   q1 = sin(2pi * m1)              (ACT Sin via free scale, fp16 out)
#   prod = q1 * q2                  (GPSIMD)
#   outT[cblk] += A_j.T @ prod      (PE, reduction over k with a-coeffs)
# Host: transpose/pad/scale/cast prep of x and params; final transpose of outT.

D = 256
C = 1000
K = 4
CK = C * K          # 4000
CKP = 4096          # padded
NCORES = 8
N = 16384
NS = N // NCORES    # 2048 per core
NH = 1024           # n-span per step (2 psum banks)
TWO_PI = float(2 * np.pi)
MAGIC = 12582912.0  # 1.5 * 2**23: fp32 RNE round-to-int trick
_CACHE = {}


def _dedupe_ldweights(nc, mybir):
    """Drop PE Ldweights that reload the exact weights already resident
    (no waits/updates attached), so same-weight matmuls pipeline back to
    back instead of paying a reload + drain per matmul."""
    removed = 0
    for blk in nc.main_func.blocks:
        last_key = None
        to_remove = []
        for inst in blk.instructions:
            if isinstance(inst, mybir.InstLdweights):
                key = (
                    str(inst.ins),
                    str(inst.tile_position),
                    str(inst.perf_mode),
                    str(inst.is_transpose),
                )
                si = inst.sync_info
                clean = si is None or (len(si.on_wait) == 0 and len(si.on_update) == 0)
                if key == last_key and clean:
                    to_remove.append(inst)
                else:
                    last_key = key
            elif isinstance(inst, mybir.InstMatmult):
                pass
            elif getattr(inst, "engine", None) is not None and str(
                getattr(inst, "engine", "")
            ).endswith("PE"):
                last_key = None
        for inst in to_remove:
            blk.instructions.remove(inst)
            removed += 1
    return removed


def _register_wrap_op():
    """out = y - round(y) with y = in0 + s0 (per-partition bias), via the
    fp32 magic-number trick: k = (y + MAGIC) - MAGIC. Exact for |y| < 2^21."""
    import re

    from concourse import dve_ops as DV
    from concourse.dve_spec import C0, C1, Spec, Src0

    for o in DV.OPS:
        if o.name == "BISINE_WRAP":
            return o

    def _ref(in0, in1, s0, s1, imm2):
        y = (np.asarray(in0, np.float32) + np.asarray(s0, np.float32)).astype(
            np.float32
        )
        t = (y + np.float32(s1)).astype(np.float32)
        k = (t - np.float32(s1)).astype(np.float32)
        return (y - k).astype(np.float32)

    y = Src0 + C0
    k = (y + C1) - C1
    op = DV.DveOp("BISINE_WRAP", Spec(body=y - k, reference=_ref), subdim=False, uops_sha={})
    DV.OPS.append(op)
    DV.CUSTOM_DVE_SPECS[op.name] = op.spec
    DV._SUB_OPCODE_FOR_NAME[op.name] = DV._CUSTOM_DVE_ROW_BASE + len(DV.OPS) - 1
    for ver in ("v3", "v4"):
        try:
            op.compile(ver)
        except ValueError as e:
            m = re.findall(r'="([0-9a-f]+)"', str(e))
            assert m, e
            op.uops_sha[ver] = m[-1]
            op.compile(ver)
    return op


def _build_nc():
    import concourse.bacc as bacc
    import concourse.mybir as mybir
    import concourse.tile as tile

    SIN = mybir.ActivationFunctionType.Sin
    F16 = mybir.dt.float16
    F32 = mybir.dt.float32

    wrap_op = _register_wrap_op()
    nc = bacc.Bacc("TRN2", target_bir_lowering=False, debug=False)

    xt_d = nc.dram_tensor("xt", [D, NS], F16, kind="ExternalInput")
    w1_d = nc.dram_tensor("w1t", [D, CKP], F16, kind="ExternalInput")
    w2_d = nc.dram_tensor("w2t", [D, CKP], F16, kind="ExternalInput")
    # Partition-major layouts so each DMA line is contiguous per partition
    # (the naive (j p)->p scatter makes 4096 tiny descriptors, ~18us).
    a_d = nc.dram_tensor("acoef", [128, (CKP // 128) * 32], F16, kind="ExternalInput")
    b1_d = nc.dram_tensor("b1v", [128, CKP // 128], F32, kind="ExternalInput")
    b2_d = nc.dram_tensor("b2v", [128, CKP // 128], F32, kind="ExternalInput")
    out_d = nc.dram_tensor("outT", [CKP // 4, NS], F32, kind="ExternalOutput")

    NJ = CKP // 128  # 32 ck-blocks
    NCB = CKP // 512  # 8 c-blocks (128 c each)

    with tile.TileContext(nc) as tc:
        with (
            tc.tile_pool(name="const", bufs=1) as cp,
            tc.tile_pool(name="work", bufs=4) as wp,
            tc.tile_pool(name="prodp", bufs=7) as pp_pool,
            tc.tile_pool(name="ob", bufs=4) as obp,
            tc.tile_pool(name="up", bufs=3, space="PSUM") as up,
            tc.tile_pool(name="op", bufs=1, space="PSUM") as op,
        ):
            xt = cp.tile([128, 2, NS], F16, tag="xt")
            w1t = cp.tile([128, 2, CKP], F16, tag="w1t")
            w2t = cp.tile([128, 2, CKP], F16, tag="w2t")
            at = cp.tile([128, NJ, 32], F16, tag="at")
            b1c = cp.tile([128, NJ], F32, tag="b1c")
            b2c = cp.tile([128, NJ], F32, tag="b2c")
            zero = cp.tile([128, 1], F32, tag="zero")

            w1_r = w1_d.ap().rearrange("(c p) k -> p c k", p=128)
            w2_r = w2_d.ap().rearrange("(c p) k -> p c k", p=128)
            at_r = a_d.ap().rearrange("p (j m) -> p j m", m=32)
            xt_r = xt_d.ap().rearrange("(c p) n -> p c n", p=128)

            # Startup-critical DMAs first. DMA kicks cost ~0.6-0.8us of the
            # issuing queue's time, so: sync gets w1 + at + b + outs, gpsimd
            # gets x + w2 (its compute starts late), scalar gets none (ACT
            # sins must not be delayed). `at` is one DMA so its first LDW
            # doesn't wait on chunks queued behind the weight stream.
            j0 = slice(0, 128)
            nc.vector.memset(zero[:], 0.0)
            # The first pair consumes ALL of x (both d-chunks, all 2048
            # cols), so x leads both rings; w1[j1..3] is only needed one
            # pair (~5us) in. Staged w2 kicks go on sync (gpsimd queue time
            # feeds the prod TTs).
            nc.sync.dma_start(w1t[:, :, j0], w1_r[:, :, j0])
            nc.gpsimd.dma_start(xt[:, 0, 0:512], xt_r[:, 0, 0:512])
            nc.sync.dma_start(xt[:, 0, 512:NH], xt_r[:, 0, 512:NH])
            nc.gpsimd.dma_start(xt[:, 1, 0:NH], xt_r[:, 1, 0:NH])
            nc.gpsimd.dma_start(xt[:, 1, NH:NS], xt_r[:, 1, NH:NS])
            nc.sync.dma_start(xt[:, 0, NH:NS], xt_r[:, 0, NH:NS])
            nc.gpsimd.dma_start(w2t[:, :, j0], w2_r[:, :, j0])
            nc.sync.dma_start(b1c[:], b1_d.ap())
            nc.sync.dma_start(b2c[:], b2_d.ap())
            nc.gpsimd.dma_start(at[:], at_r[:])
            nc.gpsimd.dma_start(w2t[:, :, 128:512], w2_r[:, :, 128:512])
            nc.sync.dma_start(w1t[:, :, 128:512], w1_r[:, :, 128:512])
            nc.gpsimd.dma_start(w2t[:, :, 512:1024], w2_r[:, :, 512:1024])
            for cb in range(1, NCB):
                rest = slice(512 * cb, 512 * (cb + 1))
                nc.sync.dma_start(w1t[:, :, rest], w1_r[:, :, rest])
            # sin argument = SCALE*m with |m| <= 0.5; SCALE one ulp under 2pi
            # keeps it strictly inside the ACT Sin [-pi, pi] domain.
            SCALE = float(np.nextafter(np.float32(TWO_PI), np.float32(0.0)))

            # Reduction matmuls are deferred DELAY steps so the PE never
            # waits on the wrap -> sin -> prod chain of the current step.
            DELAY = 4
            pending = []
            ostate = {}

            def flush_one():
                cb, nh, jj, j, prod = pending.pop(0)
                if jj == 0:
                    ostate[(cb, nh)] = op.tile([128, NH], F32, tag="o_ps", name="o_ps")
                o_ps = ostate[(cb, nh)]
                po = 32 * jj
                for h in range(NH // 512):
                    c0, c1 = h * 512, (h + 1) * 512
                    nc.tensor.matmul(
                        o_ps[po : po + 32, c0:c1],
                        at[:, j, :],
                        prod[:, c0:c1],
                        start=True,
                        stop=True,
                        tile_position=(0, po),
                    )
                if jj == 3:
                    # Copy in halves (shorter ACT slices between sins), but
                    # one DMA kick per group (kicks cost ~0.8us of queue).
                    o_sb = obp.tile([128, NH], F32, tag="o_sb")
                    for h in range(2):
                        hs = slice(h * 512, (h + 1) * 512)
                        nc.scalar.copy(o_sb[:, hs], o_ps[:, hs])
                    nc.sync.dma_start(
                        out_d.ap()[128 * cb : 128 * (cb + 1), nh * NH : (nh + 1) * NH],
                        o_sb[:],
                    )
                    del ostate[(cb, nh)]

            # Paired steps: for each j, both n-halves (nh0, nh1) are
            # computed under one pair of w1 loads (u1a/u1b live together in
            # the 3-tile u psum rotation), halving w1 LDWEIGHTS. Flushes are
            # re-ordered group-sequentially (side stash) so the single
            # output-psum tile serializes cleanly, paced one per n-chunk.
            side = []
            for cb in range(NCB):
                if cb + 2 < NCB:
                    cs = slice(512 * (cb + 2), 512 * (cb + 3))
                    nc.sync.dma_start(w2t[:, :, cs], w2_r[:, :, cs])
                for jj in range(4):
                    j = 4 * cb + jj
                    jc = slice(128 * j, 128 * (j + 1))
                    uA = up.tile([128, NH], F32, tag="u")
                    uB = up.tile([128, NH], F32, tag="u")
                    for di in range(2):
                        for u, nh in ((uA, 0), (uB, 1)):
                            for h in range(2):
                                ncol = nh * NH + h * 512
                                nc.tensor.matmul(
                                    u[:, h * 512 : (h + 1) * 512],
                                    w1t[:, di, jc],
                                    xt[:, di, ncol : ncol + 512],
                                    start=(di == 0),
                                    stop=(di == 1),
                                )
                    mA = wp.tile([128, 2, NH], F32, tag="mA")
                    mB = wp.tile([128, 2, NH], F32, tag="mB")
                    nc.vector._custom_dve(
                        wrap_op, out=mA[:, 0, :], in0=uA[:], s0=b1c[:, j : j + 1], s1=MAGIC
                    )
                    nc.vector._custom_dve(
                        wrap_op, out=mB[:, 0, :], in0=uB[:], s0=b1c[:, j : j + 1], s1=MAGIC
                    )
                    q12p = wp.tile([128, 2, 2, NH], F16, tag="q12p")
                    prodp = pp_pool.tile([128, 2, NH], F16, tag="prod")
                    for nh, m12 in ((0, mA), (1, mB)):
                        uC = up.tile([128, NH], F32, tag="u")
                        for di in range(2):
                            for h in range(2):
                                ncol = nh * NH + h * 512
                                nc.tensor.matmul(
                                    uC[:, h * 512 : (h + 1) * 512],
                                    w2t[:, di, jc],
                                    xt[:, di, ncol : ncol + 512],
                                    start=(di == 0),
                                    stop=(di == 1),
                                )
                        nc.vector._custom_dve(
                            wrap_op, out=m12[:, 1, :], in0=uC[:], s0=b2c[:, j : j + 1], s1=MAGIC
                        )
                        nc.scalar.activation(
                            q12p[:, nh, :, :], m12[:], SIN, bias=zero[:], scale=SCALE
                        )
                        # One merged TT per pair (halves GpSimd instruction
                        # overhead); the final pair keeps per-half TTs so the
                        # drain chain starts one sin earlier.
                        last_pair = cb == NCB - 1 and jj >= 2
                        if last_pair:
                            nc.gpsimd.tensor_mul(
                                prodp[:, nh, :], q12p[:, nh, 0, :], q12p[:, nh, 1, :]
                            )
                        elif nh == 1:
                            nc.gpsimd.tensor_mul(
                                prodp[:], q12p[:, :, 0, :], q12p[:, :, 1, :]
                            )
                        if nh == 0:
                            pending.append((cb, 0, jj, j, prodp[:, 0, :]))
                        else:
                            side.append((cb, 1, jj, j, prodp[:, 1, :]))
                        if len(pending) > 5:
                            flush_one()
                    if jj == 3:
                        pending.extend(side)
                        side = []
            while pending:
                flush_one()

    _dedupe_ldweights(nc, mybir)
    nc.compile()
    return nc


def _prep(x, params):
    p = np.asarray(params, dtype=np.float32).reshape(C, K, 2 * D + 3)
    a = np.ascontiguousarray(p[:, :, 0]).reshape(CK)
    w1 = np.ascontiguousarray(p[:, :, 1 : 1 + D]).reshape(CK, D)
    b1 = np.ascontiguousarray(p[:, :, 1 + D]).reshape(CK)
    w2 = np.ascontiguousarray(p[:, :, 2 + D : 2 + 2 * D]).reshape(CK, D)
    b2 = np.ascontiguousarray(p[:, :, 2 + 2 * D]).reshape(CK)

    w1p = np.zeros((CKP, D), np.float32)
    w2p = np.zeros((CKP, D), np.float32)
    w1p[:CK] = w1
    w2p[:CK] = w2
    inv2pi = np.float32(1.0 / TWO_PI)
    w1t = np.ascontiguousarray(w1p.T * inv2pi).astype(np.float16)
    w2t = np.ascontiguousarray(w2p.T * inv2pi).astype(np.float16)

    # b vectors partition-major: b[p, j] = bias for ck row j*128+p.
    b1f = np.zeros(CKP, np.float32)
    b2f = np.zeros(CKP, np.float32)
    b1f[:CK] = b1 * inv2pi
    b2f[:CK] = b2 * inv2pi
    b1v = np.ascontiguousarray(b1f.reshape(CKP // 128, 128).T)
    b2v = np.ascontiguousarray(b2f.reshape(CKP // 128, 128).T)

    ap = np.zeros(CKP, np.float32)
    ap[:CK] = a
    # acoef[row, m] = ap[row] iff m == (row % 128)//4; the 32-wide output
    # lands at psum partition offset 32*(j%4) via matmul tile_position.
    # Stored partition-major: acoef2[p, j*32+m].
    pp = np.arange(CKP) % 128
    acoef = np.zeros((CKP, 32), np.float32)
    acoef[np.arange(CKP), pp // 4] = ap
    acoef = np.ascontiguousarray(
        acoef.reshape(CKP // 128, 128, 32).transpose(1, 0, 2).reshape(128, -1)
    ).astype(np.float16)

    xt = np.ascontiguousarray(np.asarray(x, dtype=np.float32).T).astype(np.float16)  # [D, N]
    return xt, w1t, w2t, acoef, b1v, b2v


def kernel(x, params):
    from concourse import bass_utils

    if "nc" not in _CACHE:
        _CACHE["nc"] = _build_nc()
    nc = _CACHE["nc"]

    xt, w1t, w2t, acoef, b1v, b2v = _prep(x, params)
    in_maps = []
    for cid in range(NCORES):
        in_maps.append(
            {
                "xt": np.ascontiguousarray(xt[:, cid * NS : (cid + 1) * NS]),
                "w1t": w1t,
                "w2t": w2t,
                "acoef": acoef,
                "b1v": b1v,
                "b2v": b2v,
            }
        )
    res = bass_utils.run_bass_kernel_spmd(nc, in_maps, core_ids=list(range(NCORES)))
    outs = [res.results[c]["outT"] for c in range(NCORES)]
    out_t = np.concatenate(outs, axis=1)  # [1024, 16384]
    return np.ascontiguousarray(out_t[:C].T)


# revision 34
# speedup vs baseline: 1.0044x; 1.0044x over previous
import sys

sys.path.insert(0, "/opt/trn_rl_repo")
import numpy as np

# nn_BisineNetwork: out[n,c] = sum_k a[c,k] * sin(x@w1[c,k]+b1[c,k]) * sin(x@w2[c,k]+b2[c,k])
# Shapes (hardcoded): x (16384, 256) f32, params (1000, 2060) f32 -> out (16384, 1000) f32.
#
# Sharding: data-parallel over batch N across 8 cores (N_shard = 2048); params
# replicated. Per-core layout is [ck, n] (c,k merged -> 4000, padded to 4096).
# W is pre-scaled by 1/2pi on host so u arrives in "turns":
#   u1 = W1blk.T @ Xshard          (PE fp16, contraction d=256 in 2 chunks, psum f32)
#   m1 = wrap(u1 + b1') in [-.5,.5] (custom DVE op: magic-number round, 1 pass)
#   q1 = sin(2pi * m1)              (ACT Sin via free scale, fp16 out)
#   prod = q1 * q2                  (GPSIMD)
#   outT[cblk] += A_j.T @ prod      (PE, reduction over k with a-coeffs)
# Host: transpose/pad/scale/cast prep of x and params; final transpose of outT.

D = 256
C = 1000
K = 4
CK = C * K          # 4000
CKP = 4096          # padded
NCORES = 8
N = 16384
NS = N // NCORES    # 2048 per core
NH = 1024           # n-span per step (2 psum banks)
TWO_PI = float(2 * np.pi)
MAGIC = 12582912.0  # 1.5 * 2**23: fp32 RNE round-to-int trick
_CACHE = {}


def _dedupe_ldweights(nc, mybir):
    """Drop PE Ldweights that reload the exact weights already resident
    (no waits/updates attached), so same-weight matmuls pipeline back to
    back instead of paying a reload + drain per matmul."""
    removed = 0
    for blk in nc.main_func.blocks:
        last_key = None
        to_remove = []
        for inst in blk.instructions:
            if isinstance(inst, mybir.InstLdweights):
                key = (
                    str(inst.ins),
                    str(inst.tile_position),
                    str(inst.perf_mode),
                    str(inst.is_transpose),
                )
                si = inst.sync_info
                clean = si is None or (len(si.on_wait) == 0 and len(si.on_update) == 0)
                if key == last_key and clean:
                    to_remove.append(inst)
                else:
                    last_key = key
            elif isinstance(inst, mybir.InstMatmult):
                pass
            elif getattr(inst, "engine", None) is not None and str(
                getattr(inst, "engine", "")
            ).endswith("PE"):
                last_key = None
        for inst in to_remove:
            blk.instructions.remove(inst)
            removed += 1
    return removed


def _register_wrap_op():
    """out = y - round(y) with y = in0 + s0 (per-partition bias), via the
    fp32 magic-number trick: k = (y + MAGIC) - MAGIC. Exact for |y| < 2^21."""
    import re

    from concourse import dve_ops as DV
    from concourse.dve_spec import C0, C1, Spec, Src0

    for o in DV.OPS:
        if o.name == "BISINE_WRAP":
            return o

    def _ref(in0, in1, s0, s1, imm2):
        y = (np.asarray(in0, np.float32) + np.asarray(s0, np.float32)).astype(
            np.float32
        )
        t = (y + np.float32(s1)).astype(np.float32)
        k = (t - np.float32(s1)).astype(np.float32)
        return (y - k).astype(np.float32)

    y = Src0 + C0
    k = (y + C1) - C1
    op = DV.DveOp("BISINE_WRAP", Spec(body=y - k, reference=_ref), subdim=False, uops_sha={})
    DV.OPS.append(op)
    DV.CUSTOM_DVE_SPECS[op.name] = op.spec
    DV._SUB_OPCODE_FOR_NAME[op.name] = DV._CUSTOM_DVE_ROW_BASE + len(DV.OPS) - 1
    for ver in ("v3", "v4"):
        try:
            op.compile(ver)
        except ValueError as e:
            m = re.findall(r'="([0-9a-f]+)"', str(e))
            assert m, e
            op.uops_sha[ver] = m[-1]
            op.compile(ver)
    return op


def _build_nc():
    import concourse.bacc as bacc
    import concourse.mybir as mybir
    import concourse.tile as tile

    SIN = mybir.ActivationFunctionType.Sin
    F16 = mybir.dt.float16
    F32 = mybir.dt.float32

    wrap_op = _register_wrap_op()
    nc = bacc.Bacc("TRN2", target_bir_lowering=False, debug=False)

    xt_d = nc.dram_tensor("xt", [D, NS], F16, kind="ExternalInput")
    w1_d = nc.dram_tensor("w1t", [D, CKP], F16, kind="ExternalInput")
    w2_d = nc.dram_tensor("w2t", [D, CKP], F16, kind="ExternalInput")
    # Partition-major layouts so each DMA line is contiguous per partition
    # (the naive (j p)->p scatter makes 4096 tiny descriptors, ~18us).
    a_d = nc.dram_tensor("acoef", [128, (CKP // 128) * 32], F16, kind="ExternalInput")
    b1_d = nc.dram_tensor("b1v", [128, CKP // 128], F32, kind="ExternalInput")
    b2_d = nc.dram_tensor("b2v", [128, CKP // 128], F32, kind="ExternalInput")
    out_d = nc.dram_tensor("outT", [CKP // 4, NS], F32, kind="ExternalOutput")

    NJ = CKP // 128  # 32 ck-blocks
    NCB = CKP // 512  # 8 c-blocks (128 c each)

    with tile.TileContext(nc) as tc:
        with (
            tc.tile_pool(name="const", bufs=1) as cp,
            tc.tile_pool(name="work", bufs=4) as wp,
            tc.tile_pool(name="prodp", bufs=7) as pp_pool,
            tc.tile_pool(name="ob", bufs=4) as obp,
            tc.tile_pool(name="up", bufs=3, space="PSUM") as up,
            tc.tile_pool(name="op", bufs=1, space="PSUM") as op,
        ):
            xt = cp.tile([128, 2, NS], F16, tag="xt")
            w1t = cp.tile([128, 2, CKP], F16, tag="w1t")
            w2t = cp.tile([128, 2, CKP], F16, tag="w2t")
            at = cp.tile([128, NJ, 32], F16, tag="at")
            b1c = cp.tile([128, NJ], F32, tag="b1c")
            b2c = cp.tile([128, NJ], F32, tag="b2c")
            zero = cp.tile([128, 1], F32, tag="zero")

            w1_r = w1_d.ap().rearrange("(c p) k -> p c k", p=128)
            w2_r = w2_d.ap().rearrange("(c p) k -> p c k", p=128)
            at_r = a_d.ap().rearrange("p (j m) -> p j m", m=32)
            xt_r = xt_d.ap().rearrange("(c p) n -> p c n", p=128)

            # Startup-critical DMAs first. DMA kicks cost ~0.6-0.8us of the
            # issuing queue's time, so: sync gets w1 + at + b + outs, gpsimd
            # gets x + w2 (its compute starts late), scalar gets none (ACT
            # sins must not be delayed). `at` is one DMA so its first LDW
            # doesn't wait on chunks queued behind the weight stream.
            j0 = slice(0, 128)
            nc.vector.memset(zero[:], 0.0)
            # The first pair consumes ALL of x (both d-chunks, all 2048
            # cols), so x leads both rings; w1[j1..3] is only needed one
            # pair (~5us) in. Staged w2 kicks go on sync (gpsimd queue time
            # feeds the prod TTs).
            nc.sync.dma_start(w1t[:, :, j0], w1_r[:, :, j0])
            nc.gpsimd.dma_start(xt[:, 0, 0:512], xt_r[:, 0, 0:512])
            nc.sync.dma_start(xt[:, 0, 512:NH], xt_r[:, 0, 512:NH])
            nc.gpsimd.dma_start(xt[:, 1, 0:NH], xt_r[:, 1, 0:NH])
            nc.gpsimd.dma_start(xt[:, 1, NH:NS], xt_r[:, 1, NH:NS])
            nc.sync.dma_start(xt[:, 0, NH:NS], xt_r[:, 0, NH:NS])
            nc.gpsimd.dma_start(w2t[:, :, j0], w2_r[:, :, j0])
            nc.sync.dma_start(b1c[:], b1_d.ap())
            nc.sync.dma_start(b2c[:], b2_d.ap())
            nc.gpsimd.dma_start(at[:], at_r[:])
            nc.gpsimd.dma_start(w2t[:, :, 128:512], w2_r[:, :, 128:512])
            nc.sync.dma_start(w1t[:, :, 128:512], w1_r[:, :, 128:512])
            nc.gpsimd.dma_start(w2t[:, :, 512:1024], w2_r[:, :, 512:1024])
            for cb in range(1, NCB):
                rest = slice(512 * cb, 512 * (cb + 1))
                nc.sync.dma_start(w1t[:, :, rest], w1_r[:, :, rest])
            # sin argument = SCALE*m with |m| <= 0.5; SCALE one ulp under 2pi
            # keeps it strictly inside the ACT Sin [-pi, pi] domain.
            SCALE = float(np.nextafter(np.float32(TWO_PI), np.float32(0.0)))

            # Reduction matmuls are deferred DELAY steps so the PE never
            # waits on the wrap -> sin -> prod chain of the current step.
            DELAY = 4
            pending = []
            ostate = {}

            def flush_one():
                cb, nh, jj, j, prod = pending.pop(0)
                if jj == 0:
                    ostate[(cb, nh)] = op.tile([128, NH], F32, tag="o_ps", name="o_ps")
                o_ps = ostate[(cb, nh)]
                po = 32 * jj
                for h in range(NH // 512):
                    c0, c1 = h * 512, (h + 1) * 512
                    nc.tensor.matmul(
                        o_ps[po : po + 32, c0:c1],
                        at[:, j, :],
                        prod[:, c0:c1],
                        start=True,
                        stop=True,
                        tile_position=(0, po),
                    )
                if jj == 3:
                    # Copy in halves (shorter ACT slices between sins), but
                    # one DMA kick per group (kicks cost ~0.8us of queue).
                    o_sb = obp.tile([128, NH], F32, tag="o_sb")
                    for h in range(2):
                        hs = slice(h * 512, (h + 1) * 512)
                        nc.scalar.copy(o_sb[:, hs], o_ps[:, hs])
                    nc.sync.dma_start(
                        out_d.ap()[128 * cb : 128 * (cb + 1), nh * NH : (nh + 1) * NH],
                        o_sb[:],
                    )
                    del ostate[(cb, nh)]

            # Paired steps: for each j, both n-halves (nh0, nh1) are
            # computed under one pair of w1 loads (u1a/u1b live together in
            # the 3-tile u psum rotation), halving w1 LDWEIGHTS. Flushes are
            # re-ordered group-sequentially (side stash) so the single
            # output-psum tile serializes cleanly, paced one per n-chunk.
            side = []
            for cb in range(NCB):
                if cb + 2 < NCB:
                    cs = slice(512 * (cb + 2), 512 * (cb + 3))
                    nc.sync.dma_start(w2t[:, :, cs], w2_r[:, :, cs])
                for jj in range(4):
                    j = 4 * cb + jj
                    jc = slice(128 * j, 128 * (j + 1))
                    uA = up.tile([128, NH], F32, tag="u")
                    uB = up.tile([128, NH], F32, tag="u")
                    for di in range(2):
                        for u, nh in ((uA, 0), (uB, 1)):
                            for h in range(2):
                                ncol = nh * NH + h * 512
                                nc.tensor.matmul(
                                    u[:, h * 512 : (h + 1) * 512],
                                    w1t[:, di, jc],
                                    xt[:, di, ncol : ncol + 512],
                                    start=(di == 0),
                                    stop=(di == 1),
                                )
                    mA = wp.tile([128, 2, NH], F32, tag="mA")
                    mB = wp.tile([128, 2, NH], F32, tag="mB")
                    nc.vector._custom_dve(
                        wrap_op, out=mA[:, 0, :], in0=uA[:], s0=b1c[:, j : j + 1], s1=MAGIC
                    )
                    nc.vector._custom_dve(
                        wrap_op, out=mB[:, 0, :], in0=uB[:], s0=b1c[:, j : j + 1], s1=MAGIC
                    )
                    q12p = wp.tile([128, 2, 2, NH], F16, tag="q12p")
                    prodp = pp_pool.tile([128, 2, NH], F16, tag="prod")
                    for nh, m12 in ((0, mA), (1, mB)):
                        uC = up.tile([128, NH], F32, tag="u")
                        for di in range(2):
                            for h in range(2):
                                ncol = nh * NH + h * 512
                                nc.tensor.matmul(
                                    uC[:, h * 512 : (h + 1) * 512],
                                    w2t[:, di, jc],
                                    xt[:, di, ncol : ncol + 512],
                                    start=(di == 0),
                                    stop=(di == 1),
                                )
                        nc.vector._custom_dve(
                            wrap_op, out=m12[:, 1, :], in0=uC[:], s0=b2c[:, j : j + 1], s1=MAGIC
                        )
                        nc.scalar.activation(
                            q12p[:, nh, :, :], m12[:], SIN, bias=zero[:], scale=SCALE
                        )
                        # One merged TT per pair (halves GpSimd instruction
                        # overhead); the final pair keeps per-half TTs so the
                        # drain chain starts one sin earlier.
                        last_pair = cb == NCB - 1 and jj >= 2
                        if last_pair:
                            # Drain chain: DVE is idle here and 3x faster
                            # than GpSimd for fp16 elementwise.
                            nc.vector.tensor_mul(
                                prodp[:, nh, :], q12p[:, nh, 0, :], q12p[:, nh, 1, :]
                            )
                        elif nh == 1:
                            nc.gpsimd.tensor_mul(
                                prodp[:], q12p[:, :, 0, :], q12p[:, :, 1, :]
                            )
                        if nh == 0:
                            pending.append((cb, 0, jj, j, prodp[:, 0, :]))
                        else:
                            side.append((cb, 1, jj, j, prodp[:, 1, :]))
                        if len(pending) > 4:
                            flush_one()
                    if jj == 3:
                        pending.extend(side)
                        side = []
            while pending:
                flush_one()

    _dedupe_ldweights(nc, mybir)
    nc.compile()
    return nc


def _prep(x, params):
    p = np.asarray(params, dtype=np.float32).reshape(C, K, 2 * D + 3)
    a = np.ascontiguousarray(p[:, :, 0]).reshape(CK)
    w1 = np.ascontiguousarray(p[:, :, 1 : 1 + D]).reshape(CK, D)
    b1 = np.ascontiguousarray(p[:, :, 1 + D]).reshape(CK)
    w2 = np.ascontiguousarray(p[:, :, 2 + D : 2 + 2 * D]).reshape(CK, D)
    b2 = np.ascontiguousarray(p[:, :, 2 + 2 * D]).reshape(CK)

    w1p = np.zeros((CKP, D), np.float32)
    w2p = np.zeros((CKP, D), np.float32)
    w1p[:CK] = w1
    w2p[:CK] = w2
    inv2pi = np.float32(1.0 / TWO_PI)
    w1t = np.ascontiguousarray(w1p.T * inv2pi).astype(np.float16)
    w2t = np.ascontiguousarray(w2p.T * inv2pi).astype(np.float16)

    # b vectors partition-major: b[p, j] = bias for ck row j*128+p.
    b1f = np.zeros(CKP, np.float32)
    b2f = np.zeros(CKP, np.float32)
    b1f[:CK] = b1 * inv2pi
    b2f[:CK] = b2 * inv2pi
    b1v = np.ascontiguousarray(b1f.reshape(CKP // 128, 128).T)
    b2v = np.ascontiguousarray(b2f.reshape(CKP // 128, 128).T)

    ap = np.zeros(CKP, np.float32)
    ap[:CK] = a
    # acoef[row, m] = ap[row] iff m == (row % 128)//4; the 32-wide output
    # lands at psum partition offset 32*(j%4) via matmul tile_position.
    # Stored partition-major: acoef2[p, j*32+m].
    pp = np.arange(CKP) % 128
    acoef = np.zeros((CKP, 32), np.float32)
    acoef[np.arange(CKP), pp // 4] = ap
    acoef = np.ascontiguousarray(
        acoef.reshape(CKP // 128, 128, 32).transpose(1, 0, 2).reshape(128, -1)
    ).astype(np.float16)

    xt = np.ascontiguousarray(np.asarray(x, dtype=np.float32).T).astype(np.float16)  # [D, N]
    return xt, w1t, w2t, acoef, b1v, b2v


def kernel(x, params):
    from concourse import bass_utils

    if "nc" not in _CACHE:
        _CACHE["nc"] = _build_nc()
    nc = _CACHE["nc"]

    xt, w1t, w2t, acoef, b1v, b2v = _prep(x, params)
    in_maps = []
    for cid in range(NCORES):
        in_maps.append(
            {
                "xt": np.ascontiguousarray(xt[:, cid * NS : (cid + 1) * NS]),
                "w1t": w1t,
                "w2t": w2t,
                "acoef": acoef,
                "b1v": b1v,
                "b2v": b2v,
            }
        )
    res = bass_utils.run_bass_kernel_spmd(nc, in_maps, core_ids=list(range(NCORES)))
    outs = [res.results[c]["outT"] for c in range(NCORES)]
    out_t = np.concatenate(outs, axis=1)  # [1024, 16384]
    return np.ascontiguousarray(out_t[:C].T)


# revision 35
# speedup vs baseline: 1.0052x; 1.0008x over previous
import sys

sys.path.insert(0, "/opt/trn_rl_repo")
import numpy as np

# nn_BisineNetwork: out[n,c] = sum_k a[c,k] * sin(x@w1[c,k]+b1[c,k]) * sin(x@w2[c,k]+b2[c,k])
# Shapes (hardcoded): x (16384, 256) f32, params (1000, 2060) f32 -> out (16384, 1000) f32.
#
# Sharding: data-parallel over batch N across 8 cores (N_shard = 2048); params
# replicated. Per-core layout is [ck, n] (c,k merged -> 4000, padded to 4096).
# W is pre-scaled by 1/2pi on host so u arrives in "turns":
#   u1 = W1blk.T @ Xshard          (PE fp16, contraction d=256 in 2 chunks, psum f32)
#   m1 = wrap(u1 + b1') in [-.5,.5] (custom DVE op: magic-number round, 1 pass)
#   q1 = sin(2pi * m1)              (ACT Sin via free scale, fp16 out)
#   prod = q1 * q2                  (GPSIMD)
#   outT[cblk] += A_j.T @ prod      (PE, reduction over k with a-coeffs)
# Host: transpose/pad/scale/cast prep of x and params; final transpose of outT.

D = 256
C = 1000
K = 4
CK = C * K          # 4000
CKP = 4096          # padded
NCORES = 8
N = 16384
NS = N // NCORES    # 2048 per core
NH = 1024           # n-span per step (2 psum banks)
TWO_PI = float(2 * np.pi)
MAGIC = 12582912.0  # 1.5 * 2**23: fp32 RNE round-to-int trick
_CACHE = {}


def _dedupe_ldweights(nc, mybir):
    """Drop PE Ldweights that reload the exact weights already resident
    (no waits/updates attached), so same-weight matmuls pipeline back to
    back instead of paying a reload + drain per matmul."""
    removed = 0
    for blk in nc.main_func.blocks:
        last_key = None
        to_remove = []
        for inst in blk.instructions:
            if isinstance(inst, mybir.InstLdweights):
                key = (
                    str(inst.ins),
                    str(inst.tile_position),
                    str(inst.perf_mode),
                    str(inst.is_transpose),
                )
                si = inst.sync_info
                clean = si is None or (len(si.on_wait) == 0 and len(si.on_update) == 0)
                if key == last_key and clean:
                    to_remove.append(inst)
                else:
                    last_key = key
            elif isinstance(inst, mybir.InstMatmult):
                pass
            elif getattr(inst, "engine", None) is not None and str(
                getattr(inst, "engine", "")
            ).endswith("PE"):
                last_key = None
        for inst in to_remove:
            blk.instructions.remove(inst)
            removed += 1
    return removed


def _register_wrap_op():
    """out = y - round(y) with y = in0 + s0 (per-partition bias), via the
    fp32 magic-number trick: k = (y + MAGIC) - MAGIC. Exact for |y| < 2^21."""
    import re

    from concourse import dve_ops as DV
    from concourse.dve_spec import C0, C1, Spec, Src0

    for o in DV.OPS:
        if o.name == "BISINE_WRAP":
            return o

    def _ref(in0, in1, s0, s1, imm2):
        y = (np.asarray(in0, np.float32) + np.asarray(s0, np.float32)).astype(
            np.float32
        )
        t = (y + np.float32(s1)).astype(np.float32)
        k = (t - np.float32(s1)).astype(np.float32)
        return (y - k).astype(np.float32)

    y = Src0 + C0
    k = (y + C1) - C1
    op = DV.DveOp("BISINE_WRAP", Spec(body=y - k, reference=_ref), subdim=False, uops_sha={})
    DV.OPS.append(op)
    DV.CUSTOM_DVE_SPECS[op.name] = op.spec
    DV._SUB_OPCODE_FOR_NAME[op.name] = DV._CUSTOM_DVE_ROW_BASE + len(DV.OPS) - 1
    for ver in ("v3", "v4"):
        try:
            op.compile(ver)
        except ValueError as e:
            m = re.findall(r'="([0-9a-f]+)"', str(e))
            assert m, e
            op.uops_sha[ver] = m[-1]
            op.compile(ver)
    return op


def _build_nc():
    import concourse.bacc as bacc
    import concourse.mybir as mybir
    import concourse.tile as tile

    SIN = mybir.ActivationFunctionType.Sin
    F16 = mybir.dt.float16
    F32 = mybir.dt.float32

    wrap_op = _register_wrap_op()
    nc = bacc.Bacc("TRN2", target_bir_lowering=False, debug=False)

    xt_d = nc.dram_tensor("xt", [D, NS], F16, kind="ExternalInput")
    w1_d = nc.dram_tensor("w1t", [D, CKP], F16, kind="ExternalInput")
    w2_d = nc.dram_tensor("w2t", [D, CKP], F16, kind="ExternalInput")
    # Partition-major layouts so each DMA line is contiguous per partition
    # (the naive (j p)->p scatter makes 4096 tiny descriptors, ~18us).
    a_d = nc.dram_tensor("acoef", [128, (CKP // 128) * 32], F16, kind="ExternalInput")
    b1_d = nc.dram_tensor("b1v", [128, CKP // 128], F32, kind="ExternalInput")
    b2_d = nc.dram_tensor("b2v", [128, CKP // 128], F32, kind="ExternalInput")
    out_d = nc.dram_tensor("outT", [CKP // 4, NS], F32, kind="ExternalOutput")

    NJ = CKP // 128  # 32 ck-blocks
    NCB = CKP // 512  # 8 c-blocks (128 c each)

    with tile.TileContext(nc) as tc:
        with (
            tc.tile_pool(name="const", bufs=1) as cp,
            tc.tile_pool(name="work", bufs=4) as wp,
            tc.tile_pool(name="prodp", bufs=7) as pp_pool,
            tc.tile_pool(name="ob", bufs=4) as obp,
            tc.tile_pool(name="up", bufs=3, space="PSUM") as up,
            tc.tile_pool(name="op", bufs=1, space="PSUM") as op,
        ):
            xt = cp.tile([128, 2, NS], F16, tag="xt")
            w1t = cp.tile([128, 2, CKP], F16, tag="w1t")
            w2t = cp.tile([128, 2, CKP], F16, tag="w2t")
            at = cp.tile([128, NJ, 32], F16, tag="at")
            b1c = cp.tile([128, NJ], F32, tag="b1c")
            b2c = cp.tile([128, NJ], F32, tag="b2c")
            zero = cp.tile([128, 1], F32, tag="zero")

            w1_r = w1_d.ap().rearrange("(c p) k -> p c k", p=128)
            w2_r = w2_d.ap().rearrange("(c p) k -> p c k", p=128)
            at_r = a_d.ap().rearrange("p (j m) -> p j m", m=32)
            xt_r = xt_d.ap().rearrange("(c p) n -> p c n", p=128)

            # Startup-critical DMAs first. DMA kicks cost ~0.6-0.8us of the
            # issuing queue's time, so: sync gets w1 + at + b + outs, gpsimd
            # gets x + w2 (its compute starts late), scalar gets none (ACT
            # sins must not be delayed). `at` is one DMA so its first LDW
            # doesn't wait on chunks queued behind the weight stream.
            j0 = slice(0, 128)
            nc.vector.memset(zero[:], 0.0)
            # The first pair consumes ALL of x (both d-chunks, all 2048
            # cols), so x leads both rings; w1[j1..3] is only needed one
            # pair (~5us) in. Staged w2 kicks go on sync (gpsimd queue time
            # feeds the prod TTs).
            nc.sync.dma_start(w1t[:, :, j0], w1_r[:, :, j0])
            nc.gpsimd.dma_start(xt[:, 0, 0:512], xt_r[:, 0, 0:512])
            nc.sync.dma_start(xt[:, 0, 512:NH], xt_r[:, 0, 512:NH])
            nc.gpsimd.dma_start(xt[:, 1, 0:NH], xt_r[:, 1, 0:NH])
            nc.sync.dma_start(xt[:, 0, NH:NS], xt_r[:, 0, NH:NS])
            nc.gpsimd.dma_start(xt[:, 1, NH : NH + 512], xt_r[:, 1, NH : NH + 512])
            nc.sync.dma_start(xt[:, 1, NH + 512 : NS], xt_r[:, 1, NH + 512 : NS])
            nc.gpsimd.dma_start(w2t[:, :, j0], w2_r[:, :, j0])
            nc.sync.dma_start(b1c[:], b1_d.ap())
            nc.sync.dma_start(b2c[:], b2_d.ap())
            nc.gpsimd.dma_start(at[:], at_r[:])
            nc.gpsimd.dma_start(w2t[:, :, 128:512], w2_r[:, :, 128:512])
            nc.sync.dma_start(w1t[:, :, 128:512], w1_r[:, :, 128:512])
            nc.gpsimd.dma_start(w2t[:, :, 512:1024], w2_r[:, :, 512:1024])
            for cb in range(1, NCB):
                rest = slice(512 * cb, 512 * (cb + 1))
                nc.sync.dma_start(w1t[:, :, rest], w1_r[:, :, rest])
            # sin argument = SCALE*m with |m| <= 0.5; SCALE one ulp under 2pi
            # keeps it strictly inside the ACT Sin [-pi, pi] domain.
            SCALE = float(np.nextafter(np.float32(TWO_PI), np.float32(0.0)))

            # Reduction matmuls are deferred DELAY steps so the PE never
            # waits on the wrap -> sin -> prod chain of the current step.
            DELAY = 4
            pending = []
            ostate = {}

            def flush_one():
                cb, nh, jj, j, prod = pending.pop(0)
                if jj == 0:
                    ostate[(cb, nh)] = op.tile([128, NH], F32, tag="o_ps", name="o_ps")
                o_ps = ostate[(cb, nh)]
                po = 32 * jj
                for h in range(NH // 512):
                    c0, c1 = h * 512, (h + 1) * 512
                    nc.tensor.matmul(
                        o_ps[po : po + 32, c0:c1],
                        at[:, j, :],
                        prod[:, c0:c1],
                        start=True,
                        stop=True,
                        tile_position=(0, po),
                    )
                if jj == 3:
                    # Copy in halves (shorter ACT slices between sins), but
                    # one DMA kick per group (kicks cost ~0.8us of queue).
                    o_sb = obp.tile([128, NH], F32, tag="o_sb")
                    for h in range(2):
                        hs = slice(h * 512, (h + 1) * 512)
                        nc.scalar.copy(o_sb[:, hs], o_ps[:, hs])
                    nc.sync.dma_start(
                        out_d.ap()[128 * cb : 128 * (cb + 1), nh * NH : (nh + 1) * NH],
                        o_sb[:],
                    )
                    del ostate[(cb, nh)]

            # Paired steps: for each j, both n-halves (nh0, nh1) are
            # computed under one pair of w1 loads (u1a/u1b live together in
            # the 3-tile u psum rotation), halving w1 LDWEIGHTS. Flushes are
            # re-ordered group-sequentially (side stash) so the single
            # output-psum tile serializes cleanly, paced one per n-chunk.
            side = []
            for cb in range(NCB):
                if cb + 2 < NCB:
                    cs = slice(512 * (cb + 2), 512 * (cb + 3))
                    nc.sync.dma_start(w2t[:, :, cs], w2_r[:, :, cs])
                for jj in range(4):
                    j = 4 * cb + jj
                    jc = slice(128 * j, 128 * (j + 1))
                    uA = up.tile([128, NH], F32, tag="u")
                    uB = up.tile([128, NH], F32, tag="u")
                    for di in range(2):
                        for u, nh in ((uA, 0), (uB, 1)):
                            for h in range(2):
                                ncol = nh * NH + h * 512
                                nc.tensor.matmul(
                                    u[:, h * 512 : (h + 1) * 512],
                                    w1t[:, di, jc],
                                    xt[:, di, ncol : ncol + 512],
                                    start=(di == 0),
                                    stop=(di == 1),
                                )
                    mA = wp.tile([128, 2, NH], F32, tag="mA")
                    mB = wp.tile([128, 2, NH], F32, tag="mB")
                    nc.vector._custom_dve(
                        wrap_op, out=mA[:, 0, :], in0=uA[:], s0=b1c[:, j : j + 1], s1=MAGIC
                    )
                    nc.vector._custom_dve(
                        wrap_op, out=mB[:, 0, :], in0=uB[:], s0=b1c[:, j : j + 1], s1=MAGIC
                    )
                    q12p = wp.tile([128, 2, 2, NH], F16, tag="q12p")
                    prodp = pp_pool.tile([128, 2, NH], F16, tag="prod")
                    for nh, m12 in ((0, mA), (1, mB)):
                        uC = up.tile([128, NH], F32, tag="u")
                        for di in range(2):
                            for h in range(2):
                                ncol = nh * NH + h * 512
                                nc.tensor.matmul(
                                    uC[:, h * 512 : (h + 1) * 512],
                                    w2t[:, di, jc],
                                    xt[:, di, ncol : ncol + 512],
                                    start=(di == 0),
                                    stop=(di == 1),
                                )
                        nc.vector._custom_dve(
                            wrap_op, out=m12[:, 1, :], in0=uC[:], s0=b2c[:, j : j + 1], s1=MAGIC
                        )
                        nc.scalar.activation(
                            q12p[:, nh, :, :], m12[:], SIN, bias=zero[:], scale=SCALE
                        )
                        # One merged TT per pair (halves GpSimd instruction
                        # overhead); the final pair keeps per-half TTs so the
                        # drain chain starts one sin earlier.
                        last_pair = cb == NCB - 1 and jj >= 2
                        first_pairs = cb == 0 and jj <= 1
                        if last_pair:
                            # Drain chain: DVE is idle here and 3x faster
                            # than GpSimd for fp16 elementwise.
                            nc.vector.tensor_mul(
                                prodp[:, nh, :], q12p[:, nh, 0, :], q12p[:, nh, 1, :]
                            )
                        elif first_pairs:
                            # Warmup: per-half TTs so the first flushes don't
                            # wait on the merged TT's sin_b dependency.
                            nc.gpsimd.tensor_mul(
                                prodp[:, nh, :], q12p[:, nh, 0, :], q12p[:, nh, 1, :]
                            )
                        elif nh == 1:
                            nc.gpsimd.tensor_mul(
                                prodp[:], q12p[:, :, 0, :], q12p[:, :, 1, :]
                            )
                        if nh == 0:
                            pending.append((cb, 0, jj, j, prodp[:, 0, :]))
                        else:
                            side.append((cb, 1, jj, j, prodp[:, 1, :]))
                        if len(pending) > 4:
                            flush_one()
                    if jj == 3:
                        pending.extend(side)
                        side = []
            while pending:
                flush_one()

    _dedupe_ldweights(nc, mybir)
    nc.compile()
    return nc


def _prep(x, params):
    p = np.asarray(params, dtype=np.float32).reshape(C, K, 2 * D + 3)
    a = np.ascontiguousarray(p[:, :, 0]).reshape(CK)
    w1 = np.ascontiguousarray(p[:, :, 1 : 1 + D]).reshape(CK, D)
    b1 = np.ascontiguousarray(p[:, :, 1 + D]).reshape(CK)
    w2 = np.ascontiguousarray(p[:, :, 2 + D : 2 + 2 * D]).reshape(CK, D)
    b2 = np.ascontiguousarray(p[:, :, 2 + 2 * D]).reshape(CK)

    w1p = np.zeros((CKP, D), np.float32)
    w2p = np.zeros((CKP, D), np.float32)
    w1p[:CK] = w1
    w2p[:CK] = w2
    inv2pi = np.float32(1.0 / TWO_PI)
    w1t = np.ascontiguousarray(w1p.T * inv2pi).astype(np.float16)
    w2t = np.ascontiguousarray(w2p.T * inv2pi).astype(np.float16)

    # b vectors partition-major: b[p, j] = bias for ck row j*128+p.
    b1f = np.zeros(CKP, np.float32)
    b2f = np.zeros(CKP, np.float32)
    b1f[:CK] = b1 * inv2pi
    b2f[:CK] = b2 * inv2pi
    b1v = np.ascontiguousarray(b1f.reshape(CKP // 128, 128).T)
    b2v = np.ascontiguousarray(b2f.reshape(CKP // 128, 128).T)

    ap = np.zeros(CKP, np.float32)
    ap[:CK] = a
    # acoef[row, m] = ap[row] iff m == (row % 128)//4; the 32-wide output
    # lands at psum partition offset 32*(j%4) via matmul tile_position.
    # Stored partition-major: acoef2[p, j*32+m].
    pp = np.arange(CKP) % 128
    acoef = np.zeros((CKP, 32), np.float32)
    acoef[np.arange(CKP), pp // 4] = ap
    acoef = np.ascontiguousarray(
        acoef.reshape(CKP // 128, 128, 32).transpose(1, 0, 2).reshape(128, -1)
    ).astype(np.float16)

    xt = np.ascontiguousarray(np.asarray(x, dtype=np.float32).T).astype(np.float16)  # [D, N]
    return xt, w1t, w2t, acoef, b1v, b2v


def kernel(x, params):
    from concourse import bass_utils

    if "nc" not in _CACHE:
        _CACHE["nc"] = _build_nc()
    nc = _CACHE["nc"]

    xt, w1t, w2t, acoef, b1v, b2v = _prep(x, params)
    in_maps = []
    for cid in range(NCORES):
        in_maps.append(
            {
                "xt": np.ascontiguousarray(xt[:, cid * NS : (cid + 1) * NS]),
                "w1t": w1t,
                "w2t": w2t,
                "acoef": acoef,
                "b1v": b1v,
                "b2v": b2v,
            }
        )
    res = bass_utils.run_bass_kernel_spmd(nc, in_maps, core_ids=list(range(NCORES)))
    outs = [res.results[c]["outT"] for c in range(NCORES)]
    out_t = np.concatenate(outs, axis=1)  # [1024, 16384]
    return np.ascontiguousarray(out_t[:C].T)
